# revision 1
# baseline (speedup 1.0000x reference)
"""nn_ContactHead Trainium2 kernel (8-core data parallel).

out = sigmoid(w2 . relu((grid_sample(feat, uv) @ reduce_w + reduce_b) @ cls_w1 + cls_b1) + cls_b2)

Everything left of the relu is linear and bilinear sampling is linear in the
features, so the channel reductions commute with the sampling; additionally the
signed w2 is folded into the combined weights with dims permuted so that
w2>=0 dims come first (P of them):
  W  = reduce_w @ cls_w1[:, perm] * w2[perm]     (1280 x 128)  [device, PE]
  bb = (reduce_b @ cls_w1 + cls_b1)[perm]*w2[perm]             [device, PE]
  logit = sum_{d<P} max(v'_d, 0) + sum_{d>=P} min(v'_d, 0) + b2
(relu(v)*w2 == max(v*w2,0) for w2>=0 and min(v*w2,0) for w2<0.)

z[d,pix] at the 1024 pixels via PE (bf16), then pre-differenced quantities
  dzx = z(x+1)-z ; dzy = z(y+1)-z ; dzxy = dzy(x+1)-dzy
  v(wx,wy) = z00 + wx*dzx + wy*(dzy + wx*dzxy)
Token rows [z00|dzy|dzx|dzxy] (1KB bf16, pixel-major) go to DRAM (PE
transpose) and come back with dma_gather (1024 int16 idx / call,
single_packet, wrapped-16 idx layout computed on host with the bilinear
weights). Blend on DVE in 2x_1P packed mode using pair-duplicated weights
(broadcast AP ends in [1,2]); the two wx multiplies (and the two follow-up
adds) are fused into double-width ops thanks to the token order. Segment
max/min on tensor_scalar (4x mode), two 2x fold-adds halve the reduce input,
tensor_reduce (1x) finishes the dot. Sigmoid+bias on ACT.

Vert layout: vert j lives at (partition j%128, column j//128).
Outputs are stored partition-major and unshuffled on host.
"""

import ml_dtypes
import numpy as np

B, C, H, W, N = 32, 1280, 32, 32, 6890
NCORES = 8
IMGS = B // NCORES          # 4 images per core
PIX = H * W                 # 1024
PPAD = 1088                 # padded pixel slots in the dims-major z tiles
NCH = C // 128              # 10 channel chunks
MID = 128
NV = 6912                   # padded verts (= 54*128)
Q = NV // 128               # 54
TOK = 512                   # token row: 4 quantities x 128 dims (bf16)
GCOLS = [8, 8, 8, 8, 8, 8, 6]          # cols per dma_gather (<=8 -> <=1024 idx)
CHUNKS = [(0, 16), (16, 16), (32, 16), (48, 6)]   # blend (col0, ncols)

_CACHE = {}


def _build(P):
    key = ("nc", P)
    if key in _CACHE:
        return _CACHE[key]

    from contextlib import ExitStack

    import concourse.bass as bass
    import concourse.tile as tile
    from concourse import bacc, mybir
    from concourse.ap import AP

    f32 = mybir.dt.float32
    bf16 = mybir.dt.bfloat16
    i16 = mybir.dt.int16
    OP = mybir.AluOpType
    ACT = mybir.ActivationFunctionType

    nc = bacc.Bacc("TRN2", target_bir_lowering=False, debug=False)

    feat_d = nc.dram_tensor("feat", [IMGS, C, PIX], bf16, kind="ExternalInput")
    wxd_d = nc.dram_tensor("wxd", [IMGS, 128, 2 * Q], bf16, kind="ExternalInput")
    wyd_d = nc.dram_tensor("wyd", [IMGS, 128, 2 * Q], bf16, kind="ExternalInput")
    idx_d = nc.dram_tensor("idx", [IMGS, 128, NV // 16], i16, kind="ExternalInput")
    rwt_d = nc.dram_tensor("rwt", [256, C], f32, kind="ExternalInput")
    cw1_d = nc.dram_tensor("cw1", [256, MID], f32, kind="ExternalInput")
    rb_d = nc.dram_tensor("rb", [256], f32, kind="ExternalInput")
    cb1_d = nc.dram_tensor("cb1", [MID], f32, kind="ExternalInput")
    cb2_d = nc.dram_tensor("cb2", [128, 1], f32, kind="ExternalInput")
    id_d = nc.dram_tensor("ident", [128, 128], bf16, kind="ExternalInput")
    ztok_d = [
        nc.dram_tensor(f"ztok{i}", [PIX, TOK], bf16) for i in range(IMGS)
    ]
    out_d = nc.dram_tensor("out", [IMGS, 128, Q], f32, kind="ExternalOutput")

    with tile.TileContext(nc) as tc, ExitStack() as ctx:
        consts = ctx.enter_context(tc.tile_pool(name="consts", bufs=1))
        prep = ctx.enter_context(tc.tile_pool(name="prep", bufs=1))
        featp = ctx.enter_context(tc.tile_pool(name="featp", bufs=2))
        zqp = ctx.enter_context(tc.tile_pool(name="zqp", bufs=8))
        gpool = ctx.enter_context(tc.tile_pool(name="gpool", bufs=2))
        tpool = ctx.enter_context(tc.tile_pool(name="tpool", bufs=2))
        irp = ctx.enter_context(tc.tile_pool(name="irp", bufs=4))
        lg = ctx.enter_context(tc.tile_pool(name="lg", bufs=2))

        # ---------------- phase 0: combined weights (PE) ----------------
        psw_ctx = ExitStack()
        psw = psw_ctx.enter_context(tc.tile_pool(name="psw", bufs=2, space="PSUM"))
        rwt_t, cw1_t = [], []
        for k in range(2):
            rt = prep.tile([128, C], f32, tag=f"rwt{k}", name=f"rwt{k}")
            nc.sync.dma_start(rt[:], rwt_d.ap()[128 * k : 128 * (k + 1), :])
            rwt_t.append(rt)
            ct = prep.tile([128, MID], f32, tag=f"cw1{k}", name=f"cw1{k}")
            nc.sync.dma_start(ct[:], cw1_d.ap()[128 * k : 128 * (k + 1), :])
            cw1_t.append(ct)

        Wt = []
        for c in range(NCH):
            pw = psw.tile([128, 128], f32, tag="pw", name=f"pw{c}")
            for k in range(2):
                nc.tensor.matmul(
                    pw[:],
                    lhsT=rwt_t[k][:, 128 * c : 128 * (c + 1)],
                    rhs=cw1_t[k][:],
                    start=(k == 0),
                    stop=(k == 1),
                )
            wt = consts.tile([128, 128], bf16, tag=f"W{c}", name=f"W{c}")
            nc.scalar.copy(wt[:], pw[:])
            Wt.append(wt)

        rb_t = prep.tile([128, 2], f32, tag="rb", name="rb")
        nc.scalar.dma_start(rb_t[:], rb_d.ap().rearrange("(k p) -> p k", p=128))
        cb1_t = prep.tile([1, MID], f32, tag="cb1", name="cb1")
        nc.scalar.dma_start(cb1_t[:], cb1_d.ap().rearrange("(one d) -> one d", one=1))
        pb = psw.tile([1, 128], f32, tag="pb", name="pb")
        for k in range(2):
            nc.tensor.matmul(
                pb[:], lhsT=rb_t[:, k : k + 1], rhs=cw1_t[k][:],
                start=(k == 0), stop=(k == 1),
            )
        brow = prep.tile([1, 128], f32, tag="brow", name="brow")
        nc.vector.tensor_tensor(out=brow[:], in0=pb[:], in1=cb1_t[:], op=OP.add)
        bbias = consts.tile([1, 128], bf16, tag="bbias", name="bbias")
        nc.scalar.copy(bbias[:], brow[:])

        ones_t = consts.tile([1, PIX], bf16, tag="ones", name="ones")
        nc.vector.memset(ones_t[:], 1.0)
        ident = consts.tile([128, 128], bf16, tag="ident", name="ident")
        nc.scalar.dma_start(ident[:], id_d.ap())
        cb2_t = consts.tile([128, 1], f32, tag="cb2", name="cb2")
        nc.scalar.dma_start(cb2_t[:], cb2_d.ap())
        psw_ctx.close()

        zps = ctx.enter_context(tc.tile_pool(name="zps", bufs=2, space="PSUM"))
        pst = ctx.enter_context(tc.tile_pool(name="pst", bufs=3, space="PSUM"))

        # host-computed bilinear weights (dup-pairs) + wrapped-16 indices,
        # loaded up front so the gather pipeline is never input-gated
        wxds, wyds, idxts = [], [], []
        for i in range(IMGS):
            wxd = irp.tile([128, 2 * Q], bf16, tag="wxd", name=f"wxd{i}")
            wyd = irp.tile([128, 2 * Q], bf16, tag="wyd", name=f"wyd{i}")
            idxt = irp.tile([128, NV // 16], i16, tag="idx", name=f"idx{i}")
            nc.scalar.dma_start(wxd[:], wxd_d.ap()[i])
            nc.scalar.dma_start(wyd[:], wyd_d.ap()[i])
            nc.scalar.dma_start(idxt[:], idx_d.ap()[i])
            wxds.append(wxd)
            wyds.append(wyd)
            idxts.append(idxt)

        for i in range(IMGS):
            wxd, wyd, idxt = wxds[i], wyds[i], idxts[i]

            # ---------------- z at pixels (PE) ----------------
            ft = featp.tile([128, NCH * PIX], bf16, tag="ft", name=f"ft{i}")
            f_i = feat_d.ap()[i]
            nc.sync.dma_start(
                ft[:],
                AP(f_i.tensor, f_i.offset,
                   [[PIX, 128], [128 * PIX, NCH], [1, PIX]]),
            )
            zp = zps.tile([128, PIX], f32, tag="zp", name=f"zp{i}")
            for ph in range(2):
                sl = slice(512 * ph, 512 * (ph + 1))
                for c in range(NCH):
                    nc.tensor.matmul(
                        zp[:, sl],
                        lhsT=Wt[c][:],
                        rhs=ft[:, PIX * c + 512 * ph : PIX * c + 512 * (ph + 1)],
                        start=(c == 0),
                        stop=False,
                        skip_group_check=True,
                    )
                nc.tensor.matmul(
                    zp[:, sl], lhsT=bbias[:], rhs=ones_t[:, sl],
                    start=False, stop=True, skip_group_check=True,
                )

            # escape + pre-differenced quantities (dims-major, bf16)
            zq = zqp.tile([128, PPAD], bf16, tag="zq", name=f"zq{i}")
            dzx = zqp.tile([128, PPAD], bf16, tag="zq", name=f"dzx{i}")
            dzy = zqp.tile([128, PPAD], bf16, tag="zq", name=f"dzy{i}")
            dzxy = zqp.tile([128, PPAD], bf16, tag="zq", name=f"dzxy{i}")
            nc.scalar.copy(zq[:, 0:PIX], zp[:])
            nc.vector.memset(zq[:, PIX:PPAD], 0.0)
            nc.vector.tensor_tensor(out=dzx[:, 0:1056], in0=zq[:, 1:1057],
                                    in1=zq[:, 0:1056], op=OP.subtract)
            nc.vector.memset(dzx[:, 1056:PPAD], 0.0)
            nc.vector.tensor_tensor(out=dzy[:, 0:1056], in0=zq[:, 32:PPAD],
                                    in1=zq[:, 0:1056], op=OP.subtract)
            nc.vector.memset(dzy[:, 1056:PPAD], 0.0)
            nc.vector.tensor_tensor(out=dzxy[:, 0:1055], in0=dzy[:, 1:1056],
                                    in1=dzy[:, 0:1055], op=OP.subtract)
            nc.vector.memset(dzxy[:, 1055:PPAD], 0.0)

            # --------- tokens [z00|dzy|dzx|dzxy] to DRAM (PE transpose) ---------
            stg = featp.tile([128, 8 * TOK], bf16, tag="stg", name=f"stg{i}")
            for b in range(8):
                pt = pst.tile([128, TOK], bf16, tag="pt", name=f"pt{i}_{b}")
                for qi, zt in enumerate((zq, dzy, dzx, dzxy)):
                    nc.tensor.transpose(
                        pt[:, 128 * qi : 128 * (qi + 1)],
                        zt[:, 128 * b : 128 * (b + 1)],
                        ident[:],
                    )
                nc.scalar.copy(stg[:, TOK * b : TOK * (b + 1)], pt[:])
            zt_i = ztok_d[i].ap()
            nc.sync.dma_start(
                AP(zt_i.tensor, zt_i.offset,
                   [[TOK, 128], [128 * TOK, 8], [1, TOK]]),
                stg[:].rearrange("p (b t) -> p b t", t=TOK),
            )

            # ---------------- gather + blend + dot ----------------
            logit = lg.tile([128, Q], f32, tag="logit", name=f"lg{i}")
            gtiles = {}
            gcol = 0
            for gi, ncols_g in enumerate(GCOLS):
                c0 = gcol
                ct0, szt = CHUNKS[[c for c, (a, s) in enumerate(CHUNKS)
                                   if a <= c0 < a + s][0]]
                if ct0 not in gtiles:
                    gtiles[ct0] = gpool.tile(
                        [128, 16 * TOK], bf16, tag="g", name=f"g{i}_{ct0}"
                    )
                gt3 = gtiles[ct0][:].rearrange("p (c t) -> p c t", t=TOK)
                nc.gpsimd.dma_gather(
                    out_ap=gt3[:, c0 - ct0 : c0 - ct0 + ncols_g, :],
                    in_ap=ztok_d[i].ap(),
                    idxs_ap=idxt[:, c0 * 8 : (c0 + ncols_g) * 8],
                    num_idxs=ncols_g * 128,
                    num_idxs_reg=ncols_g * 128,
                    elem_size=TOK,
                )
                gcol += ncols_g

            for (c0, ncl) in CHUNKS:
                gt3 = gtiles[c0][:].rearrange("p (c t) -> p c t", t=TOK)
                g3 = gt3[:, 0:ncl, :]

                def wap(wtile, npairs, c0=c0, ncl=ncl):
                    # dup-pair weight broadcast: [[p,128],[2,ncl],[0,npairs],[1,2]]
                    a = wtile[:]
                    return AP(
                        a.tensor,
                        a.offset + 2 * c0 * a.ap[-1][0],
                        [[a.ap[0][0], 128], [2 * a.ap[-1][0], ncl],
                         [0, npairs], [a.ap[-1][0], 2]],
                    )

                def pk(apv):
                    # view [..., 2n] as [..., n, 2] (same memory)
                    return apv.rearrange("p r (d2 k) -> p r d2 k", k=2)

                t1w = tpool.tile([128, ncl * 256], bf16, tag=f"t1w{ncl}",
                                 name=f"t1w_{i}_{c0}")
                t1w3 = t1w[:].rearrange("p (r d) -> p r d", d=256)
                a2w = tpool.tile([128, ncl * 256], bf16, tag=f"a2w{ncl}",
                                 name=f"a2w_{i}_{c0}")
                a2w3 = a2w[:].rearrange("p (r d) -> p r d", d=256)
                acc = tpool.tile([128, ncl * 128], bf16, tag=f"acc{ncl}",
                                 name=f"ac_{i}_{c0}")
                acc3 = acc[:].rearrange("p (r d) -> p r d", d=128)
                # [wx*dzx | wx*dzxy]  (one wide 2x op)
                nc.vector.tensor_tensor(out=pk(t1w3), in0=pk(g3[:, :, 256:512]),
                                        in1=wap(wxd, 128), op=OP.mult)
                # [z00+wx*dzx | dzy+wx*dzxy]
                nc.vector.tensor_tensor(out=a2w3, in0=g3[:, :, 0:256],
                                        in1=t1w3, op=OP.add)
                # t3 = wy * (dzy + wx*dzxy)   (reuse t1w first half)
                t3 = t1w3[:, :, 0:128]
                nc.vector.tensor_tensor(out=pk(t3), in0=pk(a2w3[:, :, 128:256]),
                                        in1=wap(wyd, 64), op=OP.mult)
                # v' = (z00+wx*dzx) + t3
                nc.vector.tensor_tensor(out=acc3, in0=a2w3[:, :, 0:128],
                                        in1=t3, op=OP.add)
                # segment rectify: max(.,0) on dims [0,P), min(.,0) on [P,128).
                # split on even boundaries so the big ops keep 4B alignment
                # and even length (2x/4x packed mode); odd leftovers run tiny.
                def rect(lo, hi, op):
                    if lo >= hi:
                        return
                    nc.vector.tensor_scalar(out=acc3[:, :, lo:hi], in0=acc3[:, :, lo:hi],
                                            scalar1=0.0, scalar2=None, op0=op)
                Pe = P & ~1
                rect(0, Pe, OP.max)
                if P & 1:
                    rect(Pe, P, OP.max)
                    rect(P, P + 1, OP.min)
                    rect(P + 1, 128, OP.min)
                else:
                    rect(P, 128, OP.min)
                # fold 128 -> 32 with 2x adds, then 1x reduce
                u64 = a2w3[:, :, 0:64]
                nc.vector.tensor_tensor(out=u64, in0=acc3[:, :, 0:64],
                                        in1=acc3[:, :, 64:128], op=OP.add)
                u32 = a2w3[:, :, 64:96]
                nc.vector.tensor_tensor(out=u32, in0=u64[:, :, 0:32],
                                        in1=u64[:, :, 32:64], op=OP.add)
                nc.vector.tensor_reduce(
                    out=logit[:, c0 : c0 + ncl].rearrange("p (r one) -> p r one", one=1),
                    in_=u32,
                    axis=mybir.AxisListType.X,
                    op=OP.add,
                )
            ostg = lg.tile([128, Q], f32, tag="ostg", name=f"os{i}")
            nc.scalar.activation(ostg[:], logit[:], ACT.Sigmoid, bias=cb2_t[:])
            nc.scalar.dma_start(out_d.ap()[i], ostg[:])

    nc.compile()
    _CACHE[key] = nc
    return nc


def _host_prep(inputs):
    feat = np.asarray(inputs["feat_map"], dtype=np.float32)
    uv = np.asarray(inputs["verts_uv"], dtype=np.float32)
    rw = np.asarray(inputs["reduce_w"], dtype=np.float32)
    rb = np.asarray(inputs["reduce_b"], dtype=np.float32)
    w1 = np.asarray(inputs["cls_w1"], dtype=np.float32)
    b1 = np.asarray(inputs["cls_b1"], dtype=np.float32)
    w2 = np.asarray(inputs["cls_w2"], dtype=np.float32)
    b2 = np.asarray(inputs["cls_b2"], dtype=np.float32)

    # fold signed w2 into cls_w1/cls_b1, dims permuted sign-sorted (P = #pos)
    perm = np.argsort(w2 < 0, kind="stable")
    P = int((w2 >= 0).sum())
    w1f = np.ascontiguousarray(w1[:, perm] * w2[perm][None, :])
    b1f = np.ascontiguousarray(b1[perm] * w2[perm])

    rwt = np.ascontiguousarray(rw.T)                      # (256, 1280)
    uvp = np.zeros((B, NV, 2), dtype=np.float32)
    uvp[:, :N, :] = uv

    # bilinear weights + base-pixel index (same clamped-floor math as device v1)
    px = (uvp[:, :, 0] + 1.0) * np.float32(15.5)
    py = (uvp[:, :, 1] + 1.0) * np.float32(15.5)
    x0 = np.clip(np.floor(px), 0.0, 30.0)
    y0 = np.clip(np.floor(py), 0.0, 30.0)
    wx = (px - x0).astype(ml_dtypes.bfloat16)             # (B, NV)
    wy = (py - y0).astype(ml_dtypes.bfloat16)
    idx = (y0 * 32 + x0).astype(np.int16)                 # (B, NV) in [0, 990]

    # vert j at (partition j%128, col j//128); dup-pair along cols
    def dup_pair(w):
        wl = w.reshape(B, Q, 128).transpose(0, 2, 1)      # (B, 128, Q)
        return np.ascontiguousarray(np.repeat(wl, 2, axis=2))   # (B, 128, 2Q)

    wxd = dup_pair(wx)
    wyd = dup_pair(wy)

    # wrapped-16 idx layout (whole image), replicated across the 8 core groups;
    # any 128-aligned window is then a contiguous col range
    wrap = idx.reshape(B, NV // 16, 16).transpose(0, 2, 1)  # (B, 16, NV/16)
    idx_w = np.ascontiguousarray(np.tile(wrap, (1, 8, 1)))  # (B, 128, NV/16)

    featr = feat.reshape(B, C, PIX).astype(ml_dtypes.bfloat16)

    shared = {
        "rwt": rwt,
        "cw1": w1f,
        "rb": rb,
        "cb1": b1f,
        "cb2": np.full((128, 1), b2[0], dtype=np.float32),
        "ident": np.eye(128, dtype=ml_dtypes.bfloat16),
    }
    in_maps = []
    for core in range(NCORES):
        sl = slice(core * IMGS, (core + 1) * IMGS)
        m = dict(shared)
        m["feat"] = np.ascontiguousarray(featr[sl])
        m["wxd"] = np.ascontiguousarray(wxd[sl])
        m["wyd"] = np.ascontiguousarray(wyd[sl])
        m["idx"] = np.ascontiguousarray(idx_w[sl])
        in_maps.append(m)
    return in_maps, P


def kernel(**inputs):
    from concourse.bass_utils import run_bass_kernel_spmd

    in_maps, P = _host_prep(inputs)
    nc = _build(P)
    res = run_bass_kernel_spmd(nc, in_maps, list(range(NCORES)))
    out = np.empty((B, N), dtype=np.float32)
    for core in range(NCORES):
        dev = res.results[core]["out"]          # (IMGS, 128, Q), vert j at (j%128, j//128)
        full = dev.transpose(0, 2, 1).reshape(IMGS, NV)
        out[core * IMGS : (core + 1) * IMGS] = full[:, :N]
    return out

